# revision 5
# baseline (speedup 1.0000x reference)
"""CapsuleLayer dynamic-routing kernel for 8 Trainium2 NeuronCores.

Problem: x[32, 2048, 16], W[1, 2048, 64, 32, 16] -> v[32, 64, 32]
  u_hat = einsum('iodk,bik->biod', W[0], x)
  3 routing iterations (softmax over out_caps, squash over out_dim).

Sharding: in_caps (i) split 8 ways (256/core).  W shard is SBUF-resident in
bf16.  Per routing pass the tensor engine recomputes u_hat tile-by-tile
(16 concurrent small matmuls via 32x32 tile_position packing); the vector
engine applies the V-weighted d-reduction (agreement/logits) and the
exp-weighted moving operand for the selector-matmul that accumulates
s_j = sum_i c_ij * u_hat directly in PSUM.  s_j is AllReduduced across the
8 cores (it is the only cross-core quantity, 256 KB); squash + softmax
bookkeeping is replicated on every core.

Routing state trick: b_ij(t) = sum_d u_hat * (v_0+...+v_{t-1}), so no
b_ij state is carried - only the accumulated V (32x2048 f32).
"""

import os
import numpy as np
import ml_dtypes

B, IC, KD, OC, OD = 32, 2048, 16, 64, 32     # batch, in_caps, in_dim, out_caps, out_dim
NCORES = 8
ICC = IC // NCORES                            # 256 in_caps per core
NJ = ICC // 8                                 # 32 j-blocks (8 i per block)
OD2 = OC * OD                                 # 2048 flattened (o, d)
NUM_ROUTES = 3

_CACHE = {}


def _build_program():
    import concourse.bacc as bacc
    import concourse.tile as tile
    import concourse.mybir as mybir

    f32 = mybir.dt.float32
    bf16 = mybir.dt.bfloat16
    ALU = mybir.AluOpType
    ACTF = mybir.ActivationFunctionType

    nc = bacc.Bacc("TRN2", target_bir_lowering=False, debug=False, num_devices=NCORES)

    WL_d = nc.dram_tensor("WL", [128, NJ * OD2], bf16, kind="ExternalInput").ap()
    xS0_d = nc.dram_tensor("xS0", [128, NJ * B], bf16, kind="ExternalInput").ap()
    xS1_d = nc.dram_tensor("xS1", [128, NJ * B], bf16, kind="ExternalInput").ap()
    SEL1_d = nc.dram_tensor("SEL1", [128, 32], f32, kind="ExternalInput").ap()
    SEL64_d = nc.dram_tensor("SEL64", [128, 32], f32, kind="ExternalInput").ap()
    vout_d = nc.dram_tensor("v_out", [B, OD2], f32, kind="ExternalOutput").ap()

    with tile.TileContext(nc) as tc:
        with (
            tc.tile_pool(name="const", bufs=1) as cp,
            tc.tile_pool(name="work", bufs=2) as wp,
            tc.tile_pool(name="small", bufs=2) as sp,
            tc.tile_pool(name="psum", bufs=1, space="PSUM") as pp,
            tc.tile_pool(name="dram", bufs=1, space="DRAM") as dp,
        ):
            # ---- resident inputs ----
            wl = cp.tile([128, NJ * OD2], bf16, tag="wl")
            for blk in range(8):
                w = NJ * OD2 // 8
                nc.sync.dma_start(out=wl[:, blk * w:(blk + 1) * w],
                                  in_=WL_d[:, blk * w:(blk + 1) * w])
            xs = [cp.tile([128, NJ * B], bf16, tag=f"xs{s}", name=f"xs{s}") for s in range(2)]
            nc.sync.dma_start(out=xs[0][:, :], in_=xS0_d[:, :])
            nc.sync.dma_start(out=xs[1][:, :], in_=xS1_d[:, :])
            sel1 = cp.tile([128, 32], f32, tag="sel1")
            nc.sync.dma_start(out=sel1[:, :], in_=SEL1_d[:, :])
            sel64 = cp.tile([128, 32], f32, tag="sel64")
            nc.sync.dma_start(out=sel64[:, :], in_=SEL64_d[:, :])

            # ---- persistent state ----
            V4 = cp.tile([128, OD2], f32, tag="V4")    # V replicated x4 part-groups
            Vacc = cp.tile([B, OD2], f32, tag="Vacc")  # running sum of v_t

            ar_in = [dp.tile([B, OD2], f32, tag=f"ari{t}", name=f"ari{t}") for t in range(NUM_ROUTES)]
            ar_out = [dp.tile([B, OD2], f32, tag=f"aro{t}", name=f"aro{t}") for t in range(NUM_ROUTES)]

            def uhat_mms(dst_tiles, jj, s_, start, stop):
                """16 matmuls producing u_hat for i_local = jj*8 + 4*s_ + {0..3}.
                dst_tiles[ch][32r:32r+32, :512] <- u_hat[i(r), b, od-chunk ch]."""
                for ch in range(4):
                    for r in range(4):
                        nc.tensor.matmul(
                            dst_tiles[ch][32 * r:32 * r + 32, :],
                            lhsT=xs[s_][32 * r:32 * r + 32, jj * B:(jj + 1) * B],
                            rhs=wl[32 * r:32 * r + 32,
                                   jj * OD2 + ch * 512: jj * OD2 + (ch + 1) * 512],
                            start=start, stop=stop,
                            tile_position=(32 * r, 32 * r),
                        )

            def allreduce_s(t, src_psum):
                """Evacuate s (psum [32, 2048]) -> allreduce -> s_sb."""
                s_sb = wp.tile([B, OD2], f32, tag="tmp2", name=f"s_sb{t}")
                nc.vector.tensor_copy(s_sb[:, :], src_psum[0:B, :])
                nc.sync.dma_start(out=ar_in[t][:, :], in_=s_sb[:, :])
                nc.gpsimd.collective_compute(
                    "AllReduce", ALU.add,
                    replica_groups=[list(range(NCORES))],
                    ins=[ar_in[t].opt()],
                    outs=[ar_out[t].opt()],
                )
                nc.sync.dma_start(out=s_sb[:, :], in_=ar_out[t][:, :])
                return s_sb

            def squash(t, s_sb):
                """v_t = squash(s_sb).  t<2: Vacc += v_t, V4 <- replicate(Vacc).
                t==2: DMA v_t to output."""
                sq = wp.tile([B, OD2], f32, tag="tmp", name=f"sq{t}")
                nc.vector.tensor_mul(sq[:, :], s_sb[:, :], s_sb[:, :])
                n2 = sp.tile([B, OC], f32, tag="n2")
                nc.vector.tensor_reduce(
                    n2[:, :], sq[:, :].rearrange("p (o d) -> p o d", d=OD),
                    axis=mybir.AxisListType.X, op=ALU.add)
                r0 = sp.tile([B, OC], f32, tag="r0")
                nc.scalar.activation(r0[:, :], n2[:, :], ACTF.Sqrt)
                # Newton polish: n = 0.5 * (r0 + n2 / r0)
                t1 = sp.tile([B, OC], f32, tag="t1")
                nc.vector.reciprocal(t1[:, :], r0[:, :])
                nc.vector.tensor_mul(t1[:, :], t1[:, :], n2[:, :])
                t2 = sp.tile([B, OC], f32, tag="t2")
                nc.vector.tensor_add(t2[:, :], t1[:, :], r0[:, :])
                nn = sp.tile([B, OC], f32, tag="nn")
                nc.vector.tensor_scalar_mul(nn[:, :], t2[:, :], 0.5)   # |s|
                den = sp.tile([B, OC], f32, tag="den")
                nc.vector.tensor_scalar_add(den[:, :], n2[:, :], 1.0)
                rec = sp.tile([B, OC], f32, tag="rec")
                nc.vector.reciprocal(rec[:, :], den[:, :])
                qq = sp.tile([B, OC], f32, tag="qq")
                nc.vector.tensor_mul(qq[:, :], nn[:, :], rec[:, :])  # |s|/(1+|s|^2)
                vt = wp.tile([B, OD2], f32, tag="tmp", name=f"vt{t}")
                nc.vector.tensor_tensor(
                    out=vt[:, :].rearrange("p (o d) -> p o d", d=OD),
                    in0=s_sb[:, :].rearrange("p (o d) -> p o d", d=OD),
                    in1=qq[:, :].unsqueeze(2).broadcast_to([B, OC, OD]),
                    op=ALU.mult)
                if t == NUM_ROUTES - 1:
                    nc.sync.dma_start(out=vout_d[:, :], in_=vt[:, :])
                else:
                    if t == 0:
                        nc.vector.tensor_copy(Vacc[:, :], vt[:, :])
                    else:
                        nc.vector.tensor_add(Vacc[:, :], Vacc[:, :], vt[:, :])
                    for g in range(4):
                        nc.sync.dma_start(out=V4[32 * g:32 * g + 32, :], in_=Vacc[:, :])

            # ======== pass 1: s0 = sum_i u_hat / 64 ========
            accs = [pp.tile([128, 512], f32, tag=f"acc{ch}", name=f"acc{ch}") for ch in range(4)]
            for jj in range(NJ):
                for s_ in range(2):
                    uhat_mms(accs, jj, s_,
                             start=(jj == 0 and s_ == 0),
                             stop=(jj == NJ - 1 and s_ == 1))
            S0p = wp.tile([128, OD2], f32, tag="tmp", name="S0p")
            for ch in range(4):
                nc.scalar.copy(S0p[:, ch * 512:(ch + 1) * 512], accs[ch][:, :])
            sacc = pp.tile([B, OD2], f32, tag="sacc")
            for ch in range(4):
                nc.tensor.matmul(
                    sacc[0:B, ch * 512:(ch + 1) * 512], lhsT=sel64[:, :],
                    rhs=S0p[:, ch * 512:(ch + 1) * 512],
                    start=True, stop=True, tile_position=(0, 0))
            s_sb = allreduce_s(0, sacc)
            squash(0, s_sb)

            # ======== passes 2..3: fused agreement/softmax/s ========
            for t in range(1, NUM_ROUTES):
                sacc = pp.tile([B, OD2], f32, tag="sacc")
                for q in range(2 * NJ):
                    jj, s_ = divmod(q, 2)
                    uh = [pp.tile([128, 512], f32, tag=f"acc{ch}", name=f"uh{t}_{q}_{ch}") for ch in range(4)]
                    uhat_mms(uh, jj, s_, start=True, stop=True)
                    tmp = wp.tile([128, OD2], f32, tag="tmp")
                    for ch in range(4):
                        nc.vector.tensor_mul(
                            tmp[:, ch * 512:(ch + 1) * 512], uh[ch][:, :],
                            V4[:, ch * 512:(ch + 1) * 512])
                    agr = sp.tile([128, OC], f32, tag="agr")
                    nc.vector.tensor_reduce(
                        agr[:, :], tmp[:, :].rearrange("p (o d) -> p o d", d=OD),
                        axis=mybir.AxisListType.X, op=ALU.add)
                    eB = sp.tile([128, OC], f32, tag="eB")
                    nc.scalar.activation(eB[:, :], agr[:, :], ACTF.Exp)
                    Zs = sp.tile([128, 1], f32, tag="Zs")
                    nc.vector.tensor_reduce(
                        Zs[:, :], eB[:, :], axis=mybir.AxisListType.X, op=ALU.add)
                    rZ = sp.tile([128, 1], f32, tag="rZ")
                    nc.vector.reciprocal(rZ[:, :], Zs[:, :])
                    tmp2 = wp.tile([128, OD2], f32, tag="tmp2")
                    for ch in range(4):
                        nc.vector.scalar_tensor_tensor(
                            out=tmp2[:, ch * 512:(ch + 1) * 512].rearrange(
                                "p (o d) -> p o d", d=OD),
                            in0=uh[ch][:, :].rearrange("p (o d) -> p o d", d=OD),
                            scalar=rZ[:, :],
                            in1=eB[:, ch * 16:(ch + 1) * 16].unsqueeze(2)
                                .broadcast_to([128, 16, OD]),
                            op0=ALU.mult, op1=ALU.mult)
                    for ch in range(4):
                        nc.tensor.matmul(
                            sacc[0:B, ch * 512:(ch + 1) * 512], lhsT=sel1[:, :],
                            rhs=tmp2[:, ch * 512:(ch + 1) * 512],
                            start=(q == 0), stop=(q == 2 * NJ - 1),
                            tile_position=(0, 0))
                s_sb = allreduce_s(t, sacc)
                squash(t, s_sb)

    nc.compile()
    return nc


def _host_inputs(x, W):
    """Build per-core input maps (host-side relayout, not device time)."""
    W0 = np.asarray(W)[0]                       # [IC, OC, OD, KD]
    x = np.asarray(x)                           # [B, IC, KD]
    in_maps = []
    sel1 = np.zeros((128, 32), np.float32)
    for p in range(128):
        sel1[p, p % 32] = 1.0
    sel64 = sel1 / float(OC)
    for c in range(NCORES):
        Wc = W0[c * ICC:(c + 1) * ICC].reshape(NJ, 2, 4, OD2, KD)   # [j, s, r, od, k]
        WL = np.ascontiguousarray(Wc.transpose(2, 1, 4, 0, 3)       # [r, s, k, j, od]
                                  ).reshape(128, NJ * OD2)
        xc = x[:, c * ICC:(c + 1) * ICC, :].reshape(B, NJ, 2, 4, KD)  # [b, j, s, r, k]
        xss = []
        for s in range(2):
            Xs = np.zeros((4, 2, KD, NJ, B), np.float32)            # [r, s', k, j, b]
            Xs[:, s] = xc[:, :, s].transpose(2, 3, 1, 0)            # [r, k, j, b]
            xss.append(Xs.reshape(128, NJ * B))
        in_maps.append({
            "WL": WL.astype(ml_dtypes.bfloat16),
            "xS0": xss[0].astype(ml_dtypes.bfloat16),
            "xS1": xss[1].astype(ml_dtypes.bfloat16),
            "SEL1": sel1,
            "SEL64": sel64,
        })
    return in_maps


def kernel(x, W, _want_trace=False):
    from concourse.bass_utils import run_bass_kernel_spmd

    if "nc" not in _CACHE:
        _CACHE["nc"] = _build_program()
    nc = _CACHE["nc"]
    in_maps = _host_inputs(x, W)
    res = run_bass_kernel_spmd(nc, in_maps, core_ids=list(range(NCORES)),
                               trace=_want_trace)
    _CACHE["last_result"] = res
    out = np.asarray(res.results[0]["v_out"], np.float32)
    return out.reshape(B, OC, OD)


# revision 7
# speedup vs baseline: 1.5707x; 1.5707x over previous
"""CapsuleLayer dynamic-routing kernel for 8 Trainium2 NeuronCores.

Problem: x[32, 2048, 16], W[1, 2048, 64, 32, 16] -> v[32, 64, 32]
  u_hat = einsum('iodk,bik->biod', W[0], x)
  3 routing iterations (softmax over out_caps, squash over out_dim).

Sharding: in_caps (i) split 8 ways (256/core).  W shard is SBUF-resident in
bf16.  Per routing pass the tensor engine recomputes u_hat tile-by-tile
(16 concurrent small matmuls via 32x32 tile_position packing); the vector
engine applies the V-weighted d-reduction (agreement/logits) and the
exp-weighted moving operand for the selector-matmul that accumulates
s_j = sum_i c_ij * u_hat directly in PSUM.  s_j is AllReduduced across the
8 cores (it is the only cross-core quantity, 256 KB); squash + softmax
bookkeeping is replicated on every core.

Routing state trick: b_ij(t) = sum_d u_hat * (v_0+...+v_{t-1}), so no
b_ij state is carried - only the accumulated V (32x2048 f32).
"""

import os
import numpy as np
import ml_dtypes

B, IC, KD, OC, OD = 32, 2048, 16, 64, 32     # batch, in_caps, in_dim, out_caps, out_dim
NCORES = 8
ICC = IC // NCORES                            # 256 in_caps per core
NJ = ICC // 8                                 # 32 j-blocks (8 i per block)
OD2 = OC * OD                                 # 2048 flattened (o, d)
NUM_ROUTES = 3

_CACHE = {}


def _build_program():
    import concourse.bacc as bacc
    import concourse.tile as tile
    import concourse.mybir as mybir

    f32 = mybir.dt.float32
    bf16 = mybir.dt.bfloat16
    ALU = mybir.AluOpType
    ACTF = mybir.ActivationFunctionType

    nc = bacc.Bacc("TRN2", target_bir_lowering=False, debug=False, num_devices=NCORES)

    WL_d = nc.dram_tensor("WL", [128, NJ * OD2], bf16, kind="ExternalInput").ap()
    xS0_d = nc.dram_tensor("xS0", [128, NJ * B], bf16, kind="ExternalInput").ap()
    xS1_d = nc.dram_tensor("xS1", [128, NJ * B], bf16, kind="ExternalInput").ap()
    SEL1_d = nc.dram_tensor("SEL1", [128, 32], bf16, kind="ExternalInput").ap()
    SEL64_d = nc.dram_tensor("SEL64", [128, 32], f32, kind="ExternalInput").ap()
    vout_d = nc.dram_tensor("v_out", [B, OD2], f32, kind="ExternalOutput").ap()

    with tile.TileContext(nc) as tc:
        with (
            tc.tile_pool(name="const", bufs=1) as cp,
            tc.tile_pool(name="work", bufs=2) as wp,
            tc.tile_pool(name="small", bufs=2) as sp,
            tc.tile_pool(name="psum", bufs=1, space="PSUM") as pp,
            tc.tile_pool(name="dram", bufs=1, space="DRAM") as dp,
        ):
            # ---- resident inputs ----
            wl = cp.tile([128, NJ * OD2], bf16, tag="wl")
            for blk in range(8):
                w = NJ * OD2 // 8
                nc.sync.dma_start(out=wl[:, blk * w:(blk + 1) * w],
                                  in_=WL_d[:, blk * w:(blk + 1) * w])
            xs = [cp.tile([128, NJ * B], bf16, tag=f"xs{s}", name=f"xs{s}") for s in range(2)]
            nc.sync.dma_start(out=xs[0][:, :], in_=xS0_d[:, :])
            nc.sync.dma_start(out=xs[1][:, :], in_=xS1_d[:, :])
            sel1 = cp.tile([128, 32], bf16, tag="sel1")
            nc.sync.dma_start(out=sel1[:, :], in_=SEL1_d[:, :])
            sel64 = cp.tile([128, 32], f32, tag="sel64")
            nc.sync.dma_start(out=sel64[:, :], in_=SEL64_d[:, :])

            # ---- persistent state ----
            V4 = cp.tile([128, OD2], f32, tag="V4")    # V replicated x4 part-groups
            Vacc = cp.tile([B, OD2], f32, tag="Vacc")  # running sum of v_t

            ar_in = [dp.tile([B, OD2], f32, tag=f"ari{t}", name=f"ari{t}") for t in range(NUM_ROUTES)]
            ar_out = [dp.tile([B, OD2], f32, tag=f"aro{t}", name=f"aro{t}") for t in range(NUM_ROUTES)]

            def uhat_mms(dst_tiles, jj, s_, start, stop):
                """16 matmuls producing u_hat for i_local = jj*8 + 4*s_ + {0..3}.
                dst_tiles[ch][32r:32r+32, :512] <- u_hat[i(r), b, od-chunk ch]."""
                for ch in range(4):
                    for r in range(4):
                        nc.tensor.matmul(
                            dst_tiles[ch][32 * r:32 * r + 32, :],
                            lhsT=xs[s_][32 * r:32 * r + 32, jj * B:(jj + 1) * B],
                            rhs=wl[32 * r:32 * r + 32,
                                   jj * OD2 + ch * 512: jj * OD2 + (ch + 1) * 512],
                            start=start, stop=stop,
                            tile_position=(32 * r, 32 * r),
                        )

            def allreduce_s(t, src_psum):
                """Evacuate s (psum [32, 2048]) -> allreduce -> s_sb."""
                s_sb = wp.tile([B, OD2], f32, tag="uhsb", name=f"s_sb{t}")
                nc.vector.tensor_copy(s_sb[:, :], src_psum[0:B, :])
                nc.sync.dma_start(out=ar_in[t][:, :], in_=s_sb[:, :])
                nc.gpsimd.collective_compute(
                    "AllReduce", ALU.add,
                    replica_groups=[list(range(NCORES))],
                    ins=[ar_in[t].opt()],
                    outs=[ar_out[t].opt()],
                )
                nc.sync.dma_start(out=s_sb[:, :], in_=ar_out[t][:, :])
                return s_sb

            def squash(t, s_sb):
                """v_t = squash(s_sb).  t<2: Vacc += v_t, V4 <- replicate(Vacc).
                t==2: DMA v_t to output."""
                sq = wp.tile([B, OD2], f32, tag="tmp", name=f"sq{t}")
                nc.vector.tensor_mul(sq[:, :], s_sb[:, :], s_sb[:, :])
                n2 = sp.tile([B, OC], f32, tag="n2")
                nc.vector.tensor_reduce(
                    n2[:, :], sq[:, :].rearrange("p (o d) -> p o d", d=OD),
                    axis=mybir.AxisListType.X, op=ALU.add)
                r0 = sp.tile([B, OC], f32, tag="r0")
                nc.scalar.activation(r0[:, :], n2[:, :], ACTF.Sqrt)
                # Newton polish: n = 0.5 * (r0 + n2 / r0)
                t1 = sp.tile([B, OC], f32, tag="t1")
                nc.vector.reciprocal(t1[:, :], r0[:, :])
                nc.vector.tensor_mul(t1[:, :], t1[:, :], n2[:, :])
                t2 = sp.tile([B, OC], f32, tag="t2")
                nc.vector.tensor_add(t2[:, :], t1[:, :], r0[:, :])
                nn = sp.tile([B, OC], f32, tag="nn")
                nc.vector.tensor_scalar_mul(nn[:, :], t2[:, :], 0.5)   # |s|
                den = sp.tile([B, OC], f32, tag="den")
                nc.vector.tensor_scalar_add(den[:, :], n2[:, :], 1.0)
                rec = sp.tile([B, OC], f32, tag="rec")
                nc.vector.reciprocal(rec[:, :], den[:, :])
                qq = sp.tile([B, OC], f32, tag="qq")
                nc.vector.tensor_mul(qq[:, :], nn[:, :], rec[:, :])  # |s|/(1+|s|^2)
                vt = wp.tile([B, OD2], f32, tag="tmp", name=f"vt{t}")
                nc.vector.tensor_tensor(
                    out=vt[:, :].rearrange("p (o d) -> p o d", d=OD),
                    in0=s_sb[:, :].rearrange("p (o d) -> p o d", d=OD),
                    in1=qq[:, :].unsqueeze(2).broadcast_to([B, OC, OD]),
                    op=ALU.mult)
                if t == NUM_ROUTES - 1:
                    nc.sync.dma_start(out=vout_d[:, :], in_=vt[:, :])
                else:
                    if t == 0:
                        nc.vector.tensor_copy(Vacc[:, :], vt[:, :])
                    else:
                        nc.vector.tensor_add(Vacc[:, :], Vacc[:, :], vt[:, :])
                    for g in range(4):
                        nc.sync.dma_start(out=V4[32 * g:32 * g + 32, :], in_=Vacc[:, :])

            # ======== pass 1: s0 = sum_i u_hat / 64 ========
            accs = [pp.tile([128, 512], f32, tag=f"acc{ch}", name=f"acc{ch}") for ch in range(4)]
            for jj in range(NJ):
                for s_ in range(2):
                    uhat_mms(accs, jj, s_,
                             start=(jj == 0 and s_ == 0),
                             stop=(jj == NJ - 1 and s_ == 1))
            S0p = wp.tile([128, OD2], f32, tag="tmp", name="S0p")
            for ch in range(4):
                nc.scalar.copy(S0p[:, ch * 512:(ch + 1) * 512], accs[ch][:, :])
            sacc = pp.tile([B, OD2], f32, tag="sacc")
            for ch in range(4):
                nc.tensor.matmul(
                    sacc[0:B, ch * 512:(ch + 1) * 512], lhsT=sel64[:, :],
                    rhs=S0p[:, ch * 512:(ch + 1) * 512],
                    start=True, stop=True, tile_position=(0, 0))
            s_sb = allreduce_s(0, sacc)
            squash(0, s_sb)

            # ======== passes 2..3: fused agreement/softmax/s ========
            for t in range(1, NUM_ROUTES):
                sacc = pp.tile([B, OD2], f32, tag="sacc")
                for q in range(2 * NJ):
                    jj, s_ = divmod(q, 2)
                    uh = [pp.tile([128, 512], f32, tag=f"acc{ch}", name=f"uh{t}_{q}_{ch}") for ch in range(4)]
                    uhat_mms(uh, jj, s_, start=True, stop=True)
                    # scalar engine evacuates u_hat to SBUF: frees the PSUM
                    # banks after ~2us so the PE starts the next quad (stays
                    # HAM-warm) while the DVE consumes this quad from SBUF.
                    uhsb = wp.tile([128, OD2], f32, tag="uhsb", name=f"uhsb{t}_{q}")
                    for ch in range(4):
                        nc.scalar.copy(uhsb[:, ch * 512:(ch + 1) * 512], uh[ch][:, :])
                    tmp = wp.tile([128, OD2], f32, tag="tmp")
                    nc.vector.tensor_mul(tmp[:, :], uhsb[:, :], V4[:, :])
                    agr = sp.tile([128, OC], f32, tag="agr")
                    nc.vector.tensor_reduce(
                        agr[:, :], tmp[:, :].rearrange("p (o d) -> p o d", d=OD),
                        axis=mybir.AxisListType.X, op=ALU.add)
                    eB = sp.tile([128, OC], f32, tag="eB")
                    nc.scalar.activation(eB[:, :], agr[:, :], ACTF.Exp)
                    Zs = sp.tile([128, 1], f32, tag="Zs")
                    nc.vector.tensor_reduce(
                        Zs[:, :], eB[:, :], axis=mybir.AxisListType.X, op=ALU.add)
                    rZ = sp.tile([128, 1], f32, tag="rZ")
                    nc.vector.reciprocal(rZ[:, :], Zs[:, :])
                    tmp2 = wp.tile([128, OD2], bf16, tag="tmp2b", name=f"tmp2b{t}_{q}")
                    nc.vector.scalar_tensor_tensor(
                        out=tmp2[:, :].rearrange("p (o d) -> p o d", d=OD),
                        in0=uhsb[:, :].rearrange("p (o d) -> p o d", d=OD),
                        scalar=rZ[:, :],
                        in1=eB[:, :].unsqueeze(2).broadcast_to([128, OC, OD]),
                        op0=ALU.mult, op1=ALU.mult)
                    for ch in range(4):
                        nc.tensor.matmul(
                            sacc[0:B, ch * 512:(ch + 1) * 512], lhsT=sel1[:, :],
                            rhs=tmp2[:, ch * 512:(ch + 1) * 512],
                            start=(q == 0), stop=(q == 2 * NJ - 1),
                            tile_position=(0, 0))
                s_sb = allreduce_s(t, sacc)
                squash(t, s_sb)

    nc.compile()
    return nc


def _host_inputs(x, W):
    """Build per-core input maps (host-side relayout, not device time)."""
    W0 = np.asarray(W)[0]                       # [IC, OC, OD, KD]
    x = np.asarray(x)                           # [B, IC, KD]
    in_maps = []
    sel1 = np.zeros((128, 32), np.float32)
    for p in range(128):
        sel1[p, p % 32] = 1.0
    sel64 = sel1 / float(OC)
    for c in range(NCORES):
        Wc = W0[c * ICC:(c + 1) * ICC].reshape(NJ, 2, 4, OD2, KD)   # [j, s, r, od, k]
        WL = np.ascontiguousarray(Wc.transpose(2, 1, 4, 0, 3)       # [r, s, k, j, od]
                                  ).reshape(128, NJ * OD2)
        xc = x[:, c * ICC:(c + 1) * ICC, :].reshape(B, NJ, 2, 4, KD)  # [b, j, s, r, k]
        xss = []
        for s in range(2):
            Xs = np.zeros((4, 2, KD, NJ, B), np.float32)            # [r, s', k, j, b]
            Xs[:, s] = xc[:, :, s].transpose(2, 3, 1, 0)            # [r, k, j, b]
            xss.append(Xs.reshape(128, NJ * B))
        in_maps.append({
            "WL": WL.astype(ml_dtypes.bfloat16),
            "xS0": xss[0].astype(ml_dtypes.bfloat16),
            "xS1": xss[1].astype(ml_dtypes.bfloat16),
            "SEL1": sel1.astype(ml_dtypes.bfloat16),
            "SEL64": sel64,
        })
    return in_maps


def kernel(x, W, _want_trace=False):
    from concourse.bass_utils import run_bass_kernel_spmd

    if "nc" not in _CACHE:
        _CACHE["nc"] = _build_program()
    nc = _CACHE["nc"]
    in_maps = _host_inputs(x, W)
    res = run_bass_kernel_spmd(nc, in_maps, core_ids=list(range(NCORES)),
                               trace=_want_trace)
    _CACHE["last_result"] = res
    out = np.asarray(res.results[0]["v_out"], np.float32)
    return out.reshape(B, OC, OD)


# revision 9
# speedup vs baseline: 1.5816x; 1.0070x over previous
"""CapsuleLayer dynamic-routing kernel for 8 Trainium2 NeuronCores.

Problem: x[32, 2048, 16], W[1, 2048, 64, 32, 16] -> v[32, 64, 32]
  u_hat = einsum('iodk,bik->biod', W[0], x)
  3 routing iterations (softmax over out_caps, squash over out_dim).

Sharding: in_caps (i) split 8 ways (256/core).  W shard is SBUF-resident in
bf16.  Per routing pass the tensor engine recomputes u_hat tile-by-tile
(16 concurrent small matmuls via 32x32 tile_position packing); the vector
engine applies the V-weighted d-reduction (agreement/logits) and the
exp-weighted moving operand for the selector-matmul that accumulates
s_j = sum_i c_ij * u_hat directly in PSUM.  s_j is AllReduduced across the
8 cores (it is the only cross-core quantity, 256 KB); squash + softmax
bookkeeping is replicated on every core.

Routing state trick: b_ij(t) = sum_d u_hat * (v_0+...+v_{t-1}), so no
b_ij state is carried - only the accumulated V (32x2048 f32).
"""

import os
import numpy as np
import ml_dtypes

B, IC, KD, OC, OD = 32, 2048, 16, 64, 32     # batch, in_caps, in_dim, out_caps, out_dim
NCORES = 8
ICC = IC // NCORES                            # 256 in_caps per core
NJ = ICC // 8                                 # 32 j-blocks (8 i per block)
OD2 = OC * OD                                 # 2048 flattened (o, d)
NUM_ROUTES = 3

_CACHE = {}


def _build_program():
    import concourse.bacc as bacc
    import concourse.tile as tile
    import concourse.mybir as mybir

    f32 = mybir.dt.float32
    bf16 = mybir.dt.bfloat16
    ALU = mybir.AluOpType
    ACTF = mybir.ActivationFunctionType

    nc = bacc.Bacc("TRN2", target_bir_lowering=False, debug=False, num_devices=NCORES)

    WL_d = nc.dram_tensor("WL", [128, NJ * OD2], bf16, kind="ExternalInput").ap()
    xS0_d = nc.dram_tensor("xS0", [128, NJ * B], bf16, kind="ExternalInput").ap()
    xS1_d = nc.dram_tensor("xS1", [128, NJ * B], bf16, kind="ExternalInput").ap()
    SEL1_d = nc.dram_tensor("SEL1", [128, 32], bf16, kind="ExternalInput").ap()
    SEL64_d = nc.dram_tensor("SEL64", [128, 32], f32, kind="ExternalInput").ap()
    vout_d = nc.dram_tensor("v_out", [B, OD2], f32, kind="ExternalOutput").ap()

    with tile.TileContext(nc) as tc:
        with (
            tc.tile_pool(name="const", bufs=1) as cp,
            tc.tile_pool(name="work", bufs=2) as wp,
            tc.tile_pool(name="small", bufs=2) as sp,
            tc.tile_pool(name="psum", bufs=1, space="PSUM") as pp,
            tc.tile_pool(name="dram", bufs=1, space="DRAM") as dp,
        ):
            # ---- resident inputs ----
            wl = cp.tile([128, NJ * OD2], bf16, tag="wl")
            for blk in range(8):
                w = NJ * OD2 // 8
                nc.sync.dma_start(out=wl[:, blk * w:(blk + 1) * w],
                                  in_=WL_d[:, blk * w:(blk + 1) * w])
            xs = [cp.tile([128, NJ * B], bf16, tag=f"xs{s}", name=f"xs{s}") for s in range(2)]
            nc.sync.dma_start(out=xs[0][:, :], in_=xS0_d[:, :])
            nc.sync.dma_start(out=xs[1][:, :], in_=xS1_d[:, :])
            sel1 = cp.tile([128, 32], bf16, tag="sel1")
            nc.sync.dma_start(out=sel1[:, :], in_=SEL1_d[:, :])
            sel64 = cp.tile([128, 32], f32, tag="sel64")
            nc.sync.dma_start(out=sel64[:, :], in_=SEL64_d[:, :])

            # ---- persistent state ----
            V4 = cp.tile([128, OD2], f32, tag="V4")    # V replicated x4 part-groups
            Vacc = cp.tile([B, OD2], f32, tag="Vacc")  # running sum of v_t

            ar_in = [dp.tile([B, OD2], f32, tag=f"ari{t}", name=f"ari{t}") for t in range(NUM_ROUTES)]
            ar_out = [dp.tile([B, OD2], f32, tag=f"aro{t}", name=f"aro{t}") for t in range(NUM_ROUTES)]

            def uhat_mms(dst_tiles, jj, s_, start, stop):
                """16 matmuls producing u_hat for i_local = jj*8 + 4*s_ + {0..3}.
                dst_tiles[ch][32r:32r+32, :512] <- u_hat[i(r), b, od-chunk ch]."""
                for ch in range(4):
                    for r in range(4):
                        nc.tensor.matmul(
                            dst_tiles[ch][32 * r:32 * r + 32, :],
                            lhsT=xs[s_][32 * r:32 * r + 32, jj * B:(jj + 1) * B],
                            rhs=wl[32 * r:32 * r + 32,
                                   jj * OD2 + ch * 512: jj * OD2 + (ch + 1) * 512],
                            start=start, stop=stop,
                            tile_position=(32 * r, 32 * r),
                        )

            def allreduce_s(t, src_psum):
                """Evacuate s (psum [32, 2048]) -> allreduce -> s_sb."""
                s_sb = cp.tile([B, OD2], f32, tag="ssb", name=f"s_sb{t}")
                nc.vector.tensor_copy(s_sb[:, :], src_psum[0:B, :])
                nc.sync.dma_start(out=ar_in[t][:, :], in_=s_sb[:, :])
                nc.gpsimd.collective_compute(
                    "AllReduce", ALU.add,
                    replica_groups=[list(range(NCORES))],
                    ins=[ar_in[t].opt()],
                    outs=[ar_out[t].opt()],
                )
                nc.sync.dma_start(out=s_sb[:, :], in_=ar_out[t][:, :])
                return s_sb

            def squash(t, s_sb):
                """v_t = squash(s_sb).  t<2: Vacc += v_t, V4 <- replicate(Vacc).
                t==2: DMA v_t to output."""
                sq = wp.tile([B, OD2], f32, tag="tmp", name=f"sq{t}")
                nc.vector.tensor_mul(sq[:, :], s_sb[:, :], s_sb[:, :])
                n2 = sp.tile([B, OC], f32, tag="n2")
                nc.vector.tensor_reduce(
                    n2[:, :], sq[:, :].rearrange("p (o d) -> p o d", d=OD),
                    axis=mybir.AxisListType.X, op=ALU.add)
                r0 = sp.tile([B, OC], f32, tag="r0")
                nc.scalar.activation(r0[:, :], n2[:, :], ACTF.Sqrt)
                # Newton polish: n = 0.5 * (r0 + n2 / r0)
                t1 = sp.tile([B, OC], f32, tag="t1")
                nc.vector.reciprocal(t1[:, :], r0[:, :])
                nc.vector.tensor_mul(t1[:, :], t1[:, :], n2[:, :])
                t2 = sp.tile([B, OC], f32, tag="t2")
                nc.vector.tensor_add(t2[:, :], t1[:, :], r0[:, :])
                nn = sp.tile([B, OC], f32, tag="nn")
                nc.vector.tensor_scalar_mul(nn[:, :], t2[:, :], 0.5)   # |s|
                den = sp.tile([B, OC], f32, tag="den")
                nc.vector.tensor_scalar_add(den[:, :], n2[:, :], 1.0)
                rec = sp.tile([B, OC], f32, tag="rec")
                nc.vector.reciprocal(rec[:, :], den[:, :])
                qq = sp.tile([B, OC], f32, tag="qq")
                nc.vector.tensor_mul(qq[:, :], nn[:, :], rec[:, :])  # |s|/(1+|s|^2)
                vt = wp.tile([B, OD2], f32, tag="tmp", name=f"vt{t}")
                nc.vector.tensor_tensor(
                    out=vt[:, :].rearrange("p (o d) -> p o d", d=OD),
                    in0=s_sb[:, :].rearrange("p (o d) -> p o d", d=OD),
                    in1=qq[:, :].unsqueeze(2).broadcast_to([B, OC, OD]),
                    op=ALU.mult)
                if t == NUM_ROUTES - 1:
                    nc.sync.dma_start(out=vout_d[:, :], in_=vt[:, :])
                else:
                    if t == 0:
                        nc.vector.tensor_copy(Vacc[:, :], vt[:, :])
                    else:
                        nc.vector.tensor_add(Vacc[:, :], Vacc[:, :], vt[:, :])
                    for g in range(4):
                        nc.sync.dma_start(out=V4[32 * g:32 * g + 32, :], in_=Vacc[:, :])

            # ======== pass 1: s0 = sum_i u_hat / 64 ========
            accs = [pp.tile([128, 512], f32, tag=f"acc{ch}", name=f"acc{ch}") for ch in range(4)]
            for jj in range(NJ):
                for s_ in range(2):
                    uhat_mms(accs, jj, s_,
                             start=(jj == 0 and s_ == 0),
                             stop=(jj == NJ - 1 and s_ == 1))
            S0p = wp.tile([128, OD2], f32, tag="tmp", name="S0p")
            for ch in range(4):
                nc.scalar.copy(S0p[:, ch * 512:(ch + 1) * 512], accs[ch][:, :])
            sacc = pp.tile([B, OD2], f32, tag="sacc")
            for ch in range(4):
                nc.tensor.matmul(
                    sacc[0:B, ch * 512:(ch + 1) * 512], lhsT=sel64[:, :],
                    rhs=S0p[:, ch * 512:(ch + 1) * 512],
                    start=True, stop=True, tile_position=(0, 0))
            s_sb = allreduce_s(0, sacc)
            squash(0, s_sb)

            # ======== passes 2..3: fused agreement/softmax/s ========
            for t in range(1, NUM_ROUTES):
                sacc = pp.tile([B, OD2], f32, tag="sacc")
                for q in range(2 * NJ):
                    jj, s_ = divmod(q, 2)
                    uh = [pp.tile([128, 512], f32, tag=f"acc{ch}", name=f"uh{t}_{q}_{ch}") for ch in range(4)]
                    uhat_mms(uh, jj, s_, start=True, stop=True)
                    # scalar engine evacuates u_hat to SBUF: frees the PSUM
                    # banks after ~2us so the PE starts the next quad (stays
                    # HAM-warm) while the DVE consumes this quad from SBUF.
                    uhsb = wp.tile([128, OD2], bf16, tag="uhb", name=f"uhsb{t}_{q}")
                    for ch in range(4):
                        nc.scalar.copy(uhsb[:, ch * 512:(ch + 1) * 512], uh[ch][:, :])
                    tmp = wp.tile([128, OD2], f32, tag="tmp")
                    nc.vector.tensor_mul(tmp[:, :], uhsb[:, :], V4[:, :])
                    agr = sp.tile([128, OC], f32, tag="agr")
                    nc.vector.tensor_reduce(
                        agr[:, :], tmp[:, :].rearrange("p (o d) -> p o d", d=OD),
                        axis=mybir.AxisListType.X, op=ALU.add)
                    eB = sp.tile([128, OC], bf16, tag="eB")
                    nc.scalar.activation(eB[:, :], agr[:, :], ACTF.Exp)
                    Zs = sp.tile([128, 1], f32, tag="Zs")
                    nc.vector.tensor_reduce(
                        Zs[:, :], eB[:, :], axis=mybir.AxisListType.X, op=ALU.add)
                    rZ = sp.tile([128, 1], f32, tag="rZ")
                    nc.vector.reciprocal(rZ[:, :], Zs[:, :])
                    tmp2 = wp.tile([128, OD2], bf16, tag="tmp2b", name=f"tmp2b{t}_{q}")
                    nc.vector.scalar_tensor_tensor(
                        out=tmp2[:, :].rearrange("p (o d) -> p o d", d=OD),
                        in0=uhsb[:, :].rearrange("p (o d) -> p o d", d=OD),
                        scalar=rZ[:, :],
                        in1=eB[:, :].unsqueeze(2).broadcast_to([128, OC, OD]),
                        op0=ALU.mult, op1=ALU.mult)
                    for ch in range(4):
                        nc.tensor.matmul(
                            sacc[0:B, ch * 512:(ch + 1) * 512], lhsT=sel1[:, :],
                            rhs=tmp2[:, ch * 512:(ch + 1) * 512],
                            start=(q == 0), stop=(q == 2 * NJ - 1),
                            tile_position=(0, 0))
                s_sb = allreduce_s(t, sacc)
                squash(t, s_sb)

    nc.compile()
    return nc


def _host_inputs(x, W):
    """Build per-core input maps (host-side relayout, not device time)."""
    W0 = np.asarray(W)[0]                       # [IC, OC, OD, KD]
    x = np.asarray(x)                           # [B, IC, KD]
    in_maps = []
    sel1 = np.zeros((128, 32), np.float32)
    for p in range(128):
        sel1[p, p % 32] = 1.0
    sel64 = sel1 / float(OC)
    for c in range(NCORES):
        Wc = W0[c * ICC:(c + 1) * ICC].reshape(NJ, 2, 4, OD2, KD)   # [j, s, r, od, k]
        WL = np.ascontiguousarray(Wc.transpose(2, 1, 4, 0, 3)       # [r, s, k, j, od]
                                  ).reshape(128, NJ * OD2)
        xc = x[:, c * ICC:(c + 1) * ICC, :].reshape(B, NJ, 2, 4, KD)  # [b, j, s, r, k]
        xss = []
        for s in range(2):
            Xs = np.zeros((4, 2, KD, NJ, B), np.float32)            # [r, s', k, j, b]
            Xs[:, s] = xc[:, :, s].transpose(2, 3, 1, 0)            # [r, k, j, b]
            xss.append(Xs.reshape(128, NJ * B))
        in_maps.append({
            "WL": WL.astype(ml_dtypes.bfloat16),
            "xS0": xss[0].astype(ml_dtypes.bfloat16),
            "xS1": xss[1].astype(ml_dtypes.bfloat16),
            "SEL1": sel1.astype(ml_dtypes.bfloat16),
            "SEL64": sel64,
        })
    return in_maps


def kernel(x, W, _want_trace=False):
    from concourse.bass_utils import run_bass_kernel_spmd

    if "nc" not in _CACHE:
        _CACHE["nc"] = _build_program()
    nc = _CACHE["nc"]
    in_maps = _host_inputs(x, W)
    res = run_bass_kernel_spmd(nc, in_maps, core_ids=list(range(NCORES)),
                               trace=_want_trace)
    _CACHE["last_result"] = res
    out = np.asarray(res.results[0]["v_out"], np.float32)
    return out.reshape(B, OC, OD)
